# revision 1
# baseline (speedup 1.0000x reference)
"""Trainium2 Bass kernel for nn_DecoderLayer (B=4, T=S=1024, D=1024, H=16, F=4096).

Sharding: 8 cores = batch (4) x sequence-half (2). Each core computes 512 output
rows of one batch. Self-attn K/V come from raw x (full batch, host-provided
transposed), cross-attn K/V from memory -- so no inter-core communication is
needed; the host scatters inputs and gathers the 8 output chunks.

Per-core dataflow (bf16 matmuls, fp32 accumulation / residual stream):
  - K^T = wk @ x^T directly from host-provided x^T (so contraction dim D is on
    partitions everywhere, no on-device transposes of big tensors).
  - Q^T = wq_eff @ ln(x)^T, ln(x) transposed on-device via PE transposes.
    LN gain and the 1/sqrt(dk) scale are folded into wq host-side.
  - Scores are computed transposed: S^T[s,t] = K @ Q^T. Softmax is max-free
    (logits are O(1) for this model family); exp on ACT with the cross-attn
    sentence bias as a per-partition activation bias; causal/pad mask applied
    multiplicatively (exp(-inf) == 0 <=> multiply by binary mask) on DVE.
  - Softmax denominators via ones-column matmuls accumulated in the same PSUM
    bank as A^T@V (rows 0..63 = O^T_head, row 64 = denominator); reciprocal on
    DVE, replicated across 64 partitions via a K=1 matmul, then fused
    normalize+evict (scalar_tensor_tensor) produces the normalized O^T.
  - Free-dim biases enter as K=1 ones-row matmul terms; per-partition biases
    ride activation eviction.
  - FFN computed as h1^T = relu(w1_eff @ ln(x)^T + b) -> h2 = h1^T.T @ w2^T.
"""

import sys

if "/opt/trn_rl_repo" not in sys.path:
    sys.path.insert(0, "/opt/trn_rl_repo")

import numpy as np

B, T, S, D, H, F = 4, 1024, 1024, 1024, 16, 4096
DK = D // H          # 64
P = 128
NCORES = 8
TC = T // 2          # 512 rows per core
NT = TC // P         # 4 row tiles per core
ND = D // P          # 8
NS = S // P          # 8
NF = F // P          # 32
DV = H * (DK + 1)    # 1040: V' width incl. per-head ones columns
NEG = np.float32(-1e9)

_CACHE = {}
_DEBUG = False


def _build(repeat=1):
    import concourse.bacc as bacc
    import concourse.bass as bass
    import concourse.tile as tile
    from concourse import mybir
    from concourse.masks import make_identity

    f32 = mybir.dt.float32
    bf16 = mybir.dt.bfloat16
    AF = mybir.ActivationFunctionType
    ALU = mybir.AluOpType
    AX = mybir.AxisListType

    nc = bacc.Bacc("TRN2", target_bir_lowering=False, debug=False,
                   num_devices=NCORES)

    # ---------------- DRAM I/O ----------------
    dt_in = {}

    def din(name, shape, dt):
        dt_in[name] = nc.dram_tensor(name, list(shape), dt, kind="ExternalInput")
        return dt_in[name]

    xT = din("xT", (D, T), bf16)            # x[b].T
    memT = din("memT", (D, S), bf16)        # memory[b].T
    x_res = din("x_res", (TC, D), f32)      # x[b, rows]  (residual stream)
    maskT = din("maskT", (S, TC), bf16)     # binary allowed-mask, transposed
    sb_ca = din("sb_ca", (P, NS), f32)      # cross exp bias per key position
    qb_sa = din("qb_sa", (P, ND), f32)      # per-partition bias for Q^T (self)
    kb_sa = din("kb_sa", (P, ND), f32)
    qb_ca = din("qb_ca", (P, ND), f32)
    kb_ca = din("kb_ca", (P, ND), f32)
    h1b = din("h1b", (P, NF), f32)          # per-partition bias for h1^T
    w_sa = {k: din(f"w{k}_sa", (D, DV if k == "v" else D), bf16)
            for k in ("q", "k", "v", "o")}
    w_ca = {k: din(f"w{k}_ca", (D, DV if k == "v" else D), bf16)
            for k in ("q", "k", "v", "o")}
    vb_sa = din("vb_sa", (1, DV), bf16)     # V-proj bias row (ones-row matmul)
    vb_ca = din("vb_ca", (1, DV), bf16)
    c_sa = din("c_sa", (1, D), bf16)        # out-proj bias row
    c_ca = din("c_ca", (1, D), bf16)
    c_ffn = din("c_ffn", (1, D), bf16)      # ffn_b2 row
    w1T = din("w1T", (D, F), bf16)
    w2T = din("w2T", (F, D), bf16)
    out = nc.dram_tensor("out", [TC, D], f32, kind="ExternalOutput")
    dbg = {}
    if _DEBUG:
        dbg["kt"] = nc.dram_tensor("dbg_kt", [ND, P, S], bf16, kind="ExternalOutput")
        dbg["v"] = nc.dram_tensor("dbg_v", [NS, P, DV], bf16, kind="ExternalOutput")
        dbg["qt"] = nc.dram_tensor("dbg_qt", [ND, P, TC], bf16, kind="ExternalOutput")
        dbg["y1t"] = nc.dram_tensor("dbg_y1t", [ND, P, TC], bf16, kind="ExternalOutput")
        dbg["at"] = nc.dram_tensor("dbg_at", [NS, P, TC], bf16, kind="ExternalOutput")
        dbg["pod"] = nc.dram_tensor("dbg_pod", [65, TC], f32, kind="ExternalOutput")
        dbg["rep"] = nc.dram_tensor("dbg_rep", [P, TC], f32, kind="ExternalOutput")
        dbg["on"] = nc.dram_tensor("dbg_on", [ND, P, TC], bf16, kind="ExternalOutput")
        dbg["x1"] = nc.dram_tensor("dbg_x1", [NT, P, D], f32, kind="ExternalOutput")

    from contextlib import ExitStack

    with tile.TileContext(nc) as tc:
        with ExitStack() as ctx:
            pool = lambda name, bufs, **kw: ctx.enter_context(
                tc.tile_pool(name=name, bufs=bufs, **kw))
            const = pool("const", 1)
            io = pool("io", 8)
            xres_p = pool("xres", 4)
            kv_p = pool("kv", 8)
            qt_p = pool("qt", 8)
            at_p = pool("at", 6)
            ot_p = pool("ot", 8)
            yy_p = pool("yy", 4)
            yt_p = pool("yt", 8)
            h1_p = pool("h1", 32)
            wp_p = pool("wp", 8)
            w1_p = pool("w1p", 8)
            w2_p = pool("w2p", 4)
            mask_p = pool("mask", 8)
            sm_p = pool("sm", 16)
            dram_p = pool("dram", 2, space="DRAM")
            ps_p = pool("ps", 5, space="PSUM")
            po_p = pool("po", 3, space="PSUM")
            rb_p = pool("rb", 2)
            # ---------------- constants ----------------
            ident = const.tile([P, P], bf16)
            make_identity(nc, ident[:])
            ones_col = const.tile([P, 1], bf16)
            nc.vector.memset(ones_col[:], 1.0)
            ones_r64 = const.tile([1, 64], f32)
            nc.vector.memset(ones_r64[:], 1.0)
            ones_r128 = const.tile([1, P], bf16)
            nc.vector.memset(ones_r128[:], 1.0)
            eps = const.tile([P, 1], f32)
            nc.vector.memset(eps[:], 1e-5)

            def load_const(name, shape, dt):
                t = const.tile(list(shape), dt, tag=name, name=name)
                nc.sync.dma_start(t[:], dt_in[name][:])
                return t

            sb_ca_sb = load_const("sb_ca", (P, NS), f32)
            qb_sa_sb = load_const("qb_sa", (P, ND), f32)
            kb_sa_sb = load_const("kb_sa", (P, ND), f32)
            qb_ca_sb = load_const("qb_ca", (P, ND), f32)
            kb_ca_sb = load_const("kb_ca", (P, ND), f32)
            h1b_sb = load_const("h1b", (P, NF), f32)
            vb_sa_sb = load_const("vb_sa", (1, DV), bf16)
            vb_ca_sb = load_const("vb_ca", (1, DV), bf16)
            c_sa_sb = load_const("c_sa", (1, D), bf16)
            c_ca_sb = load_const("c_ca", (1, D), bf16)
            c_ffn_sb = load_const("c_ffn", (1, D), bf16)

            # residual stream, fp32, updated in place through the layer
            xres = []
            for i in range(NT):
                t = xres_p.tile([P, D], f32, tag="xres", name="xres")
                nc.sync.dma_start(t[:], x_res[i * P:(i + 1) * P, :])
                xres.append(t)

            # mask tiles (self-attn only)
            mk = []
            for i in range(NS):
                t = mask_p.tile([P, TC], bf16, tag="mk", name="mk")
                nc.sync.dma_start(t[:], maskT[i * P:(i + 1) * P, :])
                mk.append(t)

            # ---------------- helpers ----------------
            def load_w(dram, tag, pool=wp_p, width=D):
                tiles = []
                for k in range(ND):
                    t = pool.tile([P, width], bf16, tag=tag, name=tag)
                    nc.sync.dma_start(t[:], dram[k * P:(k + 1) * P, :])
                    tiles.append(t)
                return tiles

            def layernorm_T(src_tiles, tag):
                """LN (stats only; gain/bias folded into weights downstream) of
                the fp32 [TC, D] residual -> bf16 normalized rows, then PE
                transpose -> yt tiles [P, TC] (D on partitions)."""
                ytiles = [yt_p.tile([P, TC], bf16, tag="yt", name="yt")
                          for _ in range(ND)]
                for i in range(NT):
                    xt = src_tiles[i]
                    stats = sm_p.tile([P, 2, 6], f32, tag="stats", name="stats")
                    mv = sm_p.tile([P, 2], f32, tag="mv", name="mv")
                    nc.vector.bn_stats(stats[:, 0, :], xt[:, 0:512])
                    nc.vector.bn_stats(stats[:, 1, :], xt[:, 512:1024])
                    nc.vector.bn_aggr(mv[:], stats[:])
                    rstd = sm_p.tile([P, 1], f32, tag="rstd", name="rstd")
                    nc.scalar.activation(rstd[:], mv[:, 1:2], AF.Sqrt,
                                         bias=eps[:], scale=1.0)
                    nc.vector.reciprocal(rstd[:], rstd[:])
                    negmr = sm_p.tile([P, 1], f32, tag="negmr", name="negmr")
                    nc.vector.scalar_tensor_tensor(
                        negmr[:], mv[:, 0:1], -1.0, rstd[:],
                        op0=ALU.mult, op1=ALU.mult)
                    xhat = yy_p.tile([P, D], bf16, tag="xhat", name="xhat")
                    nc.scalar.activation(xhat[:], xt[:], AF.Identity,
                                         bias=negmr[:], scale=rstd[:])
                    for d in range(ND):
                        pt = ps_p.tile([P, P], bf16, tag="big", name="pstp")
                        nc.tensor.transpose(pt[:],
                                            xhat[:, d * P:(d + 1) * P],
                                            ident[:])
                        nc.vector.tensor_copy(
                            ytiles[d][:, i * P:(i + 1) * P], pt[:])
                return ytiles

            def project_T(w_tiles, rhs_tiles, n_out, bias_sb, out_tag,
                          out_pool, width):
                """out^T[o, n] = w^T.T @ rhs  (contraction over D on
                partitions). rhs_tiles: ND tiles [P, width]. Output: n_out
                tiles [P, width] bf16, evicted via ACT with per-partition
                bias."""
                otiles = []
                for m in range(n_out):
                    ot = out_pool.tile([P, width], bf16, tag=out_tag, name=out_tag)
                    for n0 in range(0, width, 512):
                        pt = ps_p.tile([P, 512], f32, tag="big", name="psbig")
                        nw = min(512, width - n0)
                        for k in range(ND):
                            nc.tensor.matmul(
                                pt[:, 0:nw],
                                w_tiles[k][:, m * P:(m + 1) * P],
                                rhs_tiles[k][:, n0:n0 + nw],
                                start=(k == 0), stop=(k == ND - 1))
                        nc.scalar.activation(ot[:, n0:n0 + nw], pt[:, 0:nw],
                                             AF.Identity,
                                             bias=bias_sb[:, m:m + 1],
                                             scale=1.0)
                    otiles.append(ot)
                return otiles

            def project_V(w_tiles, src_tiles, vb_row, out_tag):
                """V'[s, 16*65]: per head 64 value columns + a ones column
                (from a zero weight column + 1.0 in the bias row). The ones
                column makes A^T-sums (softmax denominators) fall out of the
                same accumulation as A^T@V. vb enters via a K=1 ones-row
                matmul term."""
                vtiles = []
                for m in range(NS):
                    vt = kv_p.tile([P, DV], bf16, tag=out_tag, name=out_tag)
                    for n0 in range(0, DV, 512):
                        nw = min(512, DV - n0)
                        pt = ps_p.tile([P, 512], f32, tag="big", name="psbig")
                        for k in range(ND):
                            nc.tensor.matmul(
                                pt[:, 0:nw],
                                src_tiles[k][:, m * P:(m + 1) * P],
                                w_tiles[k][:, n0:n0 + nw],
                                start=(k == 0), stop=False)
                        nc.tensor.matmul(pt[:, 0:nw], ones_r128[:, 0:P],
                                         vb_row[:, n0:n0 + nw],
                                         start=False, stop=True)
                        nc.vector.tensor_copy(vt[:, n0:n0 + nw], pt[:, 0:nw])
                    vtiles.append(vt)
                return vtiles

            def attention(kt, vt, qt, exp_bias, mask_tiles, wo_tiles,
                          c_row, dump=False):
                """Transposed-scores attention. kt: ND tiles [P, S] (K^T,
                head-major rows). vt: NS tiles [P, D]. qt: ND tiles [P, TC]
                (Q^T, pre-scaled). Returns nothing; adds attn output + c_row
                into xres in place via wo."""
                onT = [ot_p.tile([P, TC], bf16, tag="onT", name="onT") for _ in range(ND)]
                for hp in range(H // 2):
                    mt = hp
                    pods = []
                    # process an even/odd head pair; their score matmuls use
                    # row groups 0 and 64 (auto tile_position) and are emitted
                    # adjacently so the PE array runs them concurrently
                    pods = [po_p.tile([65, 512], f32, tag="od", name="od")
                            for _ in range(2)]
                    for sc in range(NS):
                        pts = []
                        for e in range(2):
                            po = 64 * e
                            pt = ps_p.tile([P, 512], f32, tag="big",
                                           name="psbig")
                            nc.tensor.matmul(
                                pt[:, 0:TC],
                                kt[mt][po:po + DK, sc * P:(sc + 1) * P],
                                qt[mt][po:po + DK, :],
                                start=True, stop=True)
                            pts.append(pt)
                        for e in range(2):
                            h = 2 * hp + e
                            a = at_p.tile([P, TC], bf16, tag="at", name="at")
                            if exp_bias is not None:
                                nc.scalar.activation(
                                    a[:], pts[e][:, 0:TC], AF.Exp,
                                    bias=exp_bias[:, sc:sc + 1], scale=1.0)
                            else:
                                nc.scalar.activation(
                                    a[:], pts[e][:, 0:TC], AF.Exp,
                                    bias=0.0, scale=1.0)
                            if mask_tiles is not None:
                                nc.vector.tensor_mul(a[:], a[:],
                                                     mask_tiles[sc][:])
                            if dump and _DEBUG and h == 0:
                                nc.sync.dma_start(dbg["at"][sc], a[:])
                            # A^T@V' accumulation consumes `a` right away:
                            # rows 0..63 = O^T_head, row 64 = denominator
                            nc.tensor.matmul(
                                pods[e][0:65, 0:TC],
                                vt[sc][:, h * 65:(h + 1) * 65],
                                a[:],
                                start=(sc == 0), stop=(sc == NS - 1))
                    for e in range(2):
                        h = 2 * hp + e
                        po = 64 * e
                        pod = pods[e]
                        rep = rb_p.tile([P, TC], f32, tag="rb", name="rb")
                        nc.vector.reciprocal(rep[0:1, :], pod[64:65, 0:TC])
                        # replicate across partitions via a DRAM bounce: DRAM
                        # APs may have partition step 0 (SBUF APs may not)
                        drow = dram_p.tile([1, TC], f32, tag="drrow",
                                           name="drrow")
                        nc.sync.dma_start(drow[:], rep[0:1, :])
                        bsrc = bass.AP(tensor=drow.tensor, offset=drow.offset,
                                       ap=[[0, 64], [1, TC]])
                        nc.sync.dma_start(rep[64:P, :], bsrc)
                        if dump and _DEBUG and h == 0:
                            pcopy = rb_p.tile([P, TC], f32, tag="rb",
                                              name="rbd")
                            nc.vector.tensor_copy(pcopy[0:65, :],
                                                  pod[0:65, 0:TC])
                            nc.sync.dma_start(dbg["pod"][:], pcopy[0:65, :])
                            nc.sync.dma_start(dbg["rep"][:], rep[:])
                        # fused normalize + evict
                        nc.vector.scalar_tensor_tensor(
                            onT[mt][po:po + DK, :], pod[0:64, 0:TC], 0.0,
                            rep[64:P, :], op0=ALU.bypass, op1=ALU.mult)
                if dump and _DEBUG:
                    for k in range(ND):
                        nc.sync.dma_start(dbg["on"][k], onT[k][:])
                # out-proj + bias row + residual add into xres (in place)
                for m in range(NT):
                    for n0 in range(0, D, 512):
                        pt = ps_p.tile([P, 512], f32, tag="big", name="psbig")
                        for k in range(ND):
                            nc.tensor.matmul(
                                pt[:], onT[k][:, m * P:(m + 1) * P],
                                wo_tiles[k][:, n0:n0 + 512],
                                start=(k == 0), stop=False)
                        nc.tensor.matmul(pt[:], ones_r128[:, 0:P],
                                         c_row[:, n0:n0 + 512],
                                         start=False, stop=True)
                        nc.vector.scalar_tensor_tensor(
                            xres[m][:, n0:n0 + 512], pt[:], 0.0,
                            xres[m][:, n0:n0 + 512],
                            op0=ALU.bypass, op1=ALU.add)

            def emit():
              # ---------------- load x^T ----------------
              xT_sb = []
              for k in range(ND):
                  t = io.tile([P, T], bf16, tag="xt", name="xt")
                  nc.sync.dma_start(t[:], xT[k * P:(k + 1) * P, :])
                  xT_sb.append(t)

              # ---------------- self attention ----------------
              wk_sb = load_w(w_sa["k"], "pw")
              kt_sa = project_T(wk_sb, xT_sb, ND, kb_sa_sb, "kt", kv_p, S)
              wv_sb = load_w(w_sa["v"], "pw", width=DV)
              v_sa = project_V(wv_sb, xT_sb, vb_sa_sb, "v")
              y1t = layernorm_T(xres, "y1")
              wq_sb = load_w(w_sa["q"], "pw")
              qt_sa = project_T(wq_sb, y1t, ND, qb_sa_sb, "qt", qt_p, TC)
              wo_sb = load_w(w_sa["o"], "pw")
              if _DEBUG:
                  for k in range(ND):
                      nc.sync.dma_start(dbg["kt"][k], kt_sa[k][:])
                      nc.sync.dma_start(dbg["v"][k], v_sa[k][:])
                      nc.sync.dma_start(dbg["qt"][k], qt_sa[k][:])
                      nc.sync.dma_start(dbg["y1t"][k], y1t[k][:])
              attention(kt_sa, v_sa, qt_sa, None, mk, wo_sb, c_sa_sb, dump=True)
              if _DEBUG:
                  for m in range(NT):
                      nc.sync.dma_start(dbg["x1"][m], xres[m][:])

              # ---------------- cross attention ----------------
              memT_sb = []
              for k in range(ND):
                  t = io.tile([P, S], bf16, tag="xt", name="xt")
                  nc.sync.dma_start(t[:], memT[k * P:(k + 1) * P, :])
                  memT_sb.append(t)
              wk_sb = load_w(w_ca["k"], "pw")
              kt_ca = project_T(wk_sb, memT_sb, ND, kb_ca_sb, "kt", kv_p, S)
              wv_sb = load_w(w_ca["v"], "pw", width=DV)
              v_ca = project_V(wv_sb, memT_sb, vb_ca_sb, "v")
              y2t = layernorm_T(xres, "y2")
              wq_sb = load_w(w_ca["q"], "pw")
              qt_ca = project_T(wq_sb, y2t, ND, qb_ca_sb, "qt", qt_p, TC)
              wo_sb = load_w(w_ca["o"], "pw")
              attention(kt_ca, v_ca, qt_ca, sb_ca_sb, None, wo_sb, c_ca_sb)

              # ---------------- FFN ----------------
              y3t = layernorm_T(xres, "y3")
              h1 = []
              for fg in range(8):          # 8 groups of 4 F-tiles
                  w1g = []
                  for k in range(ND):
                      t = w1_p.tile([P, 512], bf16, tag="w1", name="w1")
                      nc.sync.dma_start(
                          t[:], w1T[k * P:(k + 1) * P, fg * 512:(fg + 1) * 512])
                      w1g.append(t)
                  for fj in range(4):
                      fm = fg * 4 + fj
                      pt = ps_p.tile([P, 512], f32, tag="big", name="psbig")
                      for k in range(ND):
                          nc.tensor.matmul(
                              pt[:, 0:TC], w1g[k][:, fj * P:(fj + 1) * P],
                              y3t[k][:], start=(k == 0), stop=(k == ND - 1))
                      ht = h1_p.tile([P, TC], bf16, tag="h1", name="h1")
                      nc.scalar.activation(ht[:], pt[:, 0:TC], AF.Relu,
                                           bias=h1b_sb[:, fm:fm + 1], scale=1.0)
                      h1.append(ht)
              for n0 in range(0, D, 512):
                  pts = [ps_p.tile([P, 512], f32, tag="big", name="psbig") for _ in range(NT)]
                  for f in range(NF):
                      wt = w2_p.tile([P, 512], bf16, tag="w2", name="w2")
                      nc.sync.dma_start(
                          wt[:], w2T[f * P:(f + 1) * P, n0:n0 + 512])
                      for m in range(NT):
                          nc.tensor.matmul(
                              pts[m][:], h1[f][:, m * P:(m + 1) * P], wt[:],
                              start=(f == 0), stop=False)
                  for m in range(NT):
                      nc.tensor.matmul(pts[m][:], ones_r128[:, 0:P],
                                       c_ffn_sb[:, n0:n0 + 512],
                                       start=False, stop=True)
                      nc.vector.scalar_tensor_tensor(
                          xres[m][:, n0:n0 + 512], pts[m][:], 0.0,
                          xres[m][:, n0:n0 + 512],
                          op0=ALU.bypass, op1=ALU.add)

              # ---------------- write out ----------------
              for m in range(NT):
                  nc.sync.dma_start(out[m * P:(m + 1) * P, :], xres[m][:])

            for _rep in range(repeat):
                emit()

    nc.compile()
    return nc


def _prep_inputs(inputs):
    from concourse import mybir
    bf16 = mybir.dt.np(mybir.dt.bfloat16)

    f = {k: np.asarray(v, dtype=np.float32) for k, v in inputs.items()
         if k not in ("trg_mask", "trg_causal_mask", "src_mask")}
    trg_mask = np.asarray(inputs["trg_mask"])          # [B,1,1,T] int32
    causal = np.asarray(inputs["trg_causal_mask"])     # [1,1,T,T] int32
    src_mask = np.asarray(inputs["src_mask"])          # [B,1,1,S] int32

    def bf(a):
        return np.ascontiguousarray(a.astype(np.float32)).astype(bf16)

    def fold_cols(v):      # [N] -> [128, N/128], col a = v[a*128:(a+1)*128]
        return np.ascontiguousarray(v.reshape(-1, P).T.astype(np.float32))

    def _ext_v(wt):        # [D, D] -> [D, DV]: insert zero col per head
        out = np.zeros((D, DV), np.float32)
        for h in range(H):
            out[:, h * 65:h * 65 + 64] = wt[:, h * 64:(h + 1) * 64]
        return out

    def _ext_vb(bv):       # [D] -> [DV]: bias + 1.0 in ones columns
        out = np.zeros(DV, np.float32)
        for h in range(H):
            out[h * 65:h * 65 + 64] = bv[h * 64:(h + 1) * 64]
            out[h * 65 + 64] = 1.0
        return out

    scale = 1.0 / np.sqrt(np.float32(DK))
    shared = {
        "wq_sa": bf((f["sa_wq"] * f["ln1_g"][None, :] * scale).T),
        "wk_sa": bf(f["sa_wk"].T),
        "wv_sa": bf(_ext_v(f["sa_wv"].T)),
        "wo_sa": bf(f["sa_wo"].T),
        "wq_ca": bf((f["ca_wq"] * f["ln2_g"][None, :] * scale).T),
        "wk_ca": bf(f["ca_wk"].T),
        "wv_ca": bf(_ext_v(f["ca_wv"].T)),
        "wo_ca": bf(f["ca_wo"].T),
        "qb_sa": fold_cols((f["ln1_b"] @ f["sa_wq"].T + f["sa_bq"]) * scale),
        "kb_sa": fold_cols(f["sa_bk"]),
        "qb_ca": fold_cols((f["ln2_b"] @ f["ca_wq"].T + f["ca_bq"]) * scale),
        "kb_ca": fold_cols(f["ca_bk"]),
        "h1b": fold_cols(f["ln3_b"] @ f["ffn_w1"].T + f["ffn_b1"]),
        "vb_sa": bf(_ext_vb(f["sa_bv"])[None, :]),
        "vb_ca": bf(_ext_vb(f["ca_bv"])[None, :]),
        "c_sa": bf(f["sa_bo"][None, :]),
        "c_ca": bf(f["ca_bo"][None, :]),
        "c_ffn": bf(f["ffn_b2"][None, :]),
        "w1T": bf((f["ffn_w1"] * f["ln3_g"][None, :]).T),
        "w2T": bf(f["ffn_w2"].T),
    }

    # allowed[t, s] = causal[t, s] & trg_mask[b, s]; transposed -> [s, t]
    allowed = (causal[0, 0] != 0).astype(np.float32)        # [T, T]
    in_maps = []
    for c in range(NCORES):
        b, h = c // 2, c % 2
        rows = slice(h * TC, (h + 1) * TC)
        m_b = allowed * (trg_mask[b, 0, 0, :] != 0).astype(np.float32)[None, :]
        sb = (np.float32(f["ca_scale"]) * f["sentence_bias"][b]
              + np.where(src_mask[b, 0, 0, :] != 0, 0.0, NEG).astype(np.float32))
        im = dict(shared)
        im["xT"] = bf(f["x"][b].T)
        im["memT"] = bf(f["memory"][b].T)
        im["x_res"] = np.ascontiguousarray(f["x"][b, rows])
        im["maskT"] = bf(m_b[rows, :].T)
        im["sb_ca"] = fold_cols(sb)
        in_maps.append(im)
    return in_maps


def kernel(**inputs):
    from concourse.bass_utils import run_bass_kernel_spmd

    if "nc" not in _CACHE:
        _CACHE["nc"] = _build()
    nc = _CACHE["nc"]

    in_maps = _prep_inputs(inputs)
    res = run_bass_kernel_spmd(nc, in_maps, core_ids=list(range(NCORES)))

    full = np.empty((B, T, D), np.float32)
    for c in range(NCORES):
        b, h = c // 2, c % 2
        full[b, h * TC:(h + 1) * TC, :] = res.results[c]["out"]
    return full



# revision 39
# speedup vs baseline: 5.6249x; 5.6249x over previous
"""Trainium2 Bass kernel for nn_DecoderLayer (B=4, T=S=1024, D=1024, H=16, F=4096).

Sharding: 8 cores = batch (4) x sequence-half (2). Each core computes 512 output
rows of one batch. Self-attn K/V come from raw x (full batch, host-provided
transposed), cross-attn K/V from memory -- so no inter-core communication is
needed; the host scatters inputs and gathers the 8 output chunks.

Per-core dataflow (bf16 matmuls, fp32 accumulation / residual stream):
  - K^T = wk @ x^T from host-provided x^T (contraction dim D on partitions
    everywhere; no on-device transposes of big tensors). Q^T = wq_eff @
    ln(x)^T with LN gain and 1/sqrt(dk) folded into wq host-side.
  - PSUM discipline: all large psum tiles are [128, 1024] spanning two banks
    ("big2"), so paired matmuls (score head-pairs, projection column chunks,
    FFN m-pairs) write the two halves of ONE ring slot. The pair then shares
    one ring dependency, whose wait the Tile scheduler elides for the second
    matmul (a matmul carrying its own semaphore wait runs isolated, ~535 ns
    vs ~330 ns pipelined for N=512).
  - Attention is hp-batched: in batch(hp), the A^T@V / denominator matmuls
    consume exp'd score tiles produced a full batch earlier, and a 1x1
    "anchor" matmul at the batch head waits on the batch's LAST a-tile --
    all following AV/den waits are elided and the sub-tile matmul pairs
    (AV: M=64 at column groups 0/64; den: M=1 at 0/32) run concurrently.
    Scores for batch hp+1 interleave, paced by the ACT exp stream.
  - Softmax is max-free (logits are O(1) for this model family); exp+mask
    run as ONE wide ACT/DVE op over the [128, 1024] score pair; denominator
    reciprocal via the fast approx DVE op, replicated across the 64 head
    partitions with a K=1 matmul, then fused normalize+evict.
  - Free-dim biases enter as K=1 ones-row matmul terms; per-partition biases
    ride (fused, 1024-wide) activation eviction.
  - FFN: h1^T = relu(w1_eff @ ln(x)^T + b) -> h2 = h1^T.T @ w2^T, h1 tiles
    packed two-per [128, 1024] SBUF tile, w2 accumulation into big2 halves.
"""

import sys

if "/opt/trn_rl_repo" not in sys.path:
    sys.path.insert(0, "/opt/trn_rl_repo")

import numpy as np

B, T, S, D, H, F = 4, 1024, 1024, 1024, 16, 4096
DK = D // H          # 64
P = 128
NCORES = 8
TC = T // 2          # 512 rows per core
NT = TC // P         # 4 row tiles per core
ND = D // P          # 8
NS = S // P          # 8
NF = F // P          # 32
NEG = np.float32(-1e9)

_CACHE = {}
_DEBUG = False


def _build(repeat=1):
    import concourse.bacc as bacc
    import concourse.bass as bass
    import concourse.tile as tile
    from concourse import mybir
    from concourse.masks import make_identity

    f32 = mybir.dt.float32
    bf16 = mybir.dt.bfloat16
    AF = mybir.ActivationFunctionType
    ALU = mybir.AluOpType

    nc = bacc.Bacc("TRN2", target_bir_lowering=False, debug=False,
                   num_devices=NCORES)

    # ---------------- DRAM I/O ----------------
    dt_in = {}

    def din(name, shape, dt):
        dt_in[name] = nc.dram_tensor(name, list(shape), dt, kind="ExternalInput")
        return dt_in[name]

    xT = din("xT", (D, T), bf16)            # x[b].T
    memT = din("memT", (D, S), bf16)        # memory[b].T
    x_res = din("x_res", (TC, D), f32)      # x[b, rows]  (residual stream)
    maskT = din("maskT", (S, TC), bf16)     # binary allowed-mask, transposed
    sb_ca = din("sb_ca", (P, NS), f32)      # cross exp bias per key position
    qb_sa = din("qb_sa", (P, ND), f32)      # per-partition bias for Q^T (self)
    kb_sa = din("kb_sa", (P, ND), f32)
    qb_ca = din("qb_ca", (P, ND), f32)
    kb_ca = din("kb_ca", (P, ND), f32)
    h1b = din("h1b", (P, NF), f32)          # per-partition bias for h1^T
    w_sa = {k: din(f"w{k}_sa", (D, D), bf16) for k in ("q", "k", "v", "o")}
    w_ca = {k: din(f"w{k}_ca", (D, D), bf16) for k in ("q", "k", "v", "o")}
    vb_sa = din("vb_sa", (1, D), bf16)      # V-proj bias row (ones-row matmul)
    vb_ca = din("vb_ca", (1, D), bf16)
    c_sa = din("c_sa", (1, D), bf16)        # out-proj bias row
    c_ca = din("c_ca", (1, D), bf16)
    c_ffn = din("c_ffn", (1, D), bf16)      # ffn_b2 row
    w1T = din("w1T", (D, F), bf16)
    w2T = din("w2T", (F, D), bf16)
    out = nc.dram_tensor("out", [TC, D], f32, kind="ExternalOutput")
    dbg = {}
    if _DEBUG:
        dbg["at"] = nc.dram_tensor("dbg_at", [NS, 2, P, TC], f32, kind="ExternalOutput")
        dbg["den"] = nc.dram_tensor("dbg_den", [P, TC], f32, kind="ExternalOutput")
        dbg["pods"] = nc.dram_tensor("dbg_pods", [P, TC], f32, kind="ExternalOutput")
        dbg["rep"] = nc.dram_tensor("dbg_rep", [P, TC], f32, kind="ExternalOutput")
        dbg["bcs"] = nc.dram_tensor("dbg_bcs", [P, TC], f32, kind="ExternalOutput")
        dbg["on"] = nc.dram_tensor("dbg_on", [ND, P, TC], f32, kind="ExternalOutput")

    from contextlib import ExitStack

    with tile.TileContext(nc) as tc:
        with ExitStack() as ctx:
            pool = lambda name, bufs, **kw: ctx.enter_context(
                tc.tile_pool(name=name, bufs=bufs, **kw))
            const = pool("const", 1)
            io = pool("io", 8)
            xres_p = pool("xres", 4)
            kv_p = pool("kv", 8)
            qt_p = pool("qt", 8)
            at_p = pool("at", 16)           # [P, 2*TC] a-tiles / h1 pairs
            ot_p = pool("ot", 8)
            yy_p = pool("yy", 3)
            yt_p = pool("yt", 8)
            wp_p = pool("wp", 16)
            w1_p = pool("w1p", 8)
            w2_p = pool("w2p", 8)
            mask_p = pool("mask", 8)
            sm_p = pool("sm", 16)
            rb_p = pool("rb", 2)
            rbb_p = pool("rbb", 2)
            dbg_p = pool("dbg", 2) if _DEBUG else None
            # PSUM: 8 banks: big2 2x2 + pods 2 + den/bc shared 2
            ps_p = pool("ps", 2, space="PSUM")
            po_p = pool("po", 2, space="PSUM")
            db_p = pool("db", 2, space="PSUM")

            def big2():
                return ps_p.tile([P, 2 * TC], f32, tag="big2", name="big2")

            # ---------------- constants ----------------
            ident = const.tile([P, P], bf16)
            make_identity(nc, ident[:])
            ones_col = const.tile([P, 1], bf16)
            nc.vector.memset(ones_col[:], 1.0)
            ones64 = const.tile([P, 64], bf16)
            nc.vector.memset(ones64[:], 1.0)
            ones_r128 = const.tile([1, P], bf16)
            nc.vector.memset(ones_r128[:], 1.0)
            eps = const.tile([P, 1], f32)
            nc.vector.memset(eps[:], 1e-5)

            def load_const(name, shape, dt):
                t = const.tile(list(shape), dt, tag=name, name=name)
                nc.sync.dma_start(t[:], dt_in[name][:])
                return t

            sb_ca_sb = load_const("sb_ca", (P, NS), f32)
            qb_sa_sb = load_const("qb_sa", (P, ND), f32)
            kb_sa_sb = load_const("kb_sa", (P, ND), f32)
            qb_ca_sb = load_const("qb_ca", (P, ND), f32)
            kb_ca_sb = load_const("kb_ca", (P, ND), f32)
            h1b_sb = load_const("h1b", (P, NF), f32)
            vb_sa_sb = load_const("vb_sa", (1, D), bf16)
            vb_ca_sb = load_const("vb_ca", (1, D), bf16)
            c_sa_sb = load_const("c_sa", (1, D), bf16)
            c_ca_sb = load_const("c_ca", (1, D), bf16)
            c_ffn_sb = load_const("c_ffn", (1, D), bf16)

            # ---------------- helpers ----------------
            def load_w(dram, tag, pool=wp_p, width=D):
                tiles = []
                for k in range(ND):
                    t = pool.tile([P, width], bf16, tag=tag, name=tag)
                    nc.sync.dma_start(t[:], dram[k * P:(k + 1) * P, :])
                    tiles.append(t)
                return tiles

            def layernorm_T(src_tiles, tag):
                """LN (stats only; gain/bias folded into weights downstream) of
                the fp32 [TC, D] residual -> bf16 normalized rows, then PE
                transpose -> yt tiles [P, TC] (D on partitions)."""
                ytiles = [yt_p.tile([P, TC], bf16, tag="yt", name="yt")
                          for _ in range(ND)]
                for i in range(NT):
                    xt = src_tiles[i]
                    stats = sm_p.tile([P, 2, 6], f32, tag="stats", name="stats")
                    mv = sm_p.tile([P, 2], f32, tag="mv", name="mv")
                    nc.vector.bn_stats(stats[:, 0, :], xt[:, 0:512])
                    nc.vector.bn_stats(stats[:, 1, :], xt[:, 512:1024])
                    nc.vector.bn_aggr(mv[:], stats[:])
                    rstd = sm_p.tile([P, 1], f32, tag="rstd", name="rstd")
                    nc.scalar.activation(rstd[:], mv[:, 1:2], AF.Sqrt,
                                         bias=eps[:], scale=1.0)
                    nc.vector.reciprocal(rstd[:], rstd[:])
                    negmr = sm_p.tile([P, 1], f32, tag="negmr", name="negmr")
                    nc.vector.scalar_tensor_tensor(
                        negmr[:], mv[:, 0:1], -1.0, rstd[:],
                        op0=ALU.mult, op1=ALU.mult)
                    xhat = yy_p.tile([P, D], bf16, tag="xhat", name="xhat")
                    nc.scalar.activation(xhat[:], xt[:], AF.Identity,
                                         bias=negmr[:], scale=rstd[:])
                    for d in range(ND):
                        pt = ps_p.tile([P, P], bf16, tag="big2", name="pstp")
                        nc.tensor.transpose(pt[:],
                                            xhat[:, d * P:(d + 1) * P],
                                            ident[:])
                        nc.vector.tensor_copy(
                            ytiles[d][:, i * P:(i + 1) * P], pt[:])
                return ytiles

            def project_T(w_tiles, rhs_tiles, n_out, bias_sb, out_tag,
                          out_pool, width, m_lo=0, otiles=None):
                """out^T[o, n] = w^T.T @ rhs  (contraction over D on
                partitions). rhs_tiles: ND tiles [P, width]. Output: n_out
                tiles [P, width] bf16, fused ACT eviction with per-partition
                bias."""
                if otiles is None:
                    otiles = []
                for m in range(m_lo, n_out):
                    ot = out_pool.tile([P, width], bf16, tag=out_tag, name=out_tag)
                    pt = big2()
                    for n0 in range(0, width, 512):
                        for k in range(ND):
                            nc.tensor.matmul(
                                pt[:, n0:n0 + 512],
                                w_tiles[k][:, m * P:(m + 1) * P],
                                rhs_tiles[k][:, n0:n0 + 512],
                                start=(k == 0), stop=(k == ND - 1))
                    nc.scalar.activation(ot[:, 0:width], pt[:, 0:width],
                                         AF.Identity,
                                         bias=bias_sb[:, m:m + 1],
                                         scale=1.0)
                    otiles.append(ot)
                return otiles

            def project_V(w_tiles, src_tiles, vb_row, out_tag):
                """V[s, d] tiles (key positions on partitions). vb enters via
                a K=1 ones-row matmul term; fused eviction on DVE."""
                vtiles = []
                for m in range(NS):
                    vt = kv_p.tile([P, D], bf16, tag=out_tag, name=out_tag)
                    pt = big2()
                    for n0 in range(0, D, 512):
                        for k in range(ND):
                            nc.tensor.matmul(
                                pt[:, n0:n0 + 512],
                                src_tiles[k][:, m * P:(m + 1) * P],
                                w_tiles[k][:, n0:n0 + 512],
                                start=(k == 0), stop=False)
                        nc.tensor.matmul(pt[:, n0:n0 + 512], ones_r128[:, 0:P],
                                         vb_row[:, n0:n0 + 512],
                                         start=False, stop=True)
                    nc.vector.tensor_copy(vt[:], pt[:])
                    vtiles.append(vt)
                return vtiles

            def dbg_dump(dram_slice, src_ap):
                t = dbg_p.tile([P, TC], mybir.dt.float32, tag="dbg", name="dbg")
                nc.vector.tensor_copy(t[:], src_ap)
                nc.sync.dma_start(dram_slice, t[:])

            def attention(kt, vt, qt, exp_bias, mask_tiles, wo_tiles, c_row,
                          dump=False, after_prologue=None):
                """hp-batched transposed-scores attention (see module doc).
                Adds attn output + c_row into xres in place via wo."""
                onT = [ot_p.tile([P, TC], bf16, tag="onT", name="onT")
                       for _ in range(ND)]

                def score_step(hp, sc):
                    """score pair (one big2 psum) -> fused exp+mask -> a2."""
                    pt = big2()
                    for e in range(2):
                        po = 64 * e
                        nc.tensor.matmul(
                            pt[:, e * TC:e * TC + TC],
                            kt[hp][po:po + DK, sc * P:(sc + 1) * P],
                            qt[hp][po:po + DK, :],
                            start=True, stop=True)
                    a2 = at_p.tile([P, 2 * TC], bf16, tag="at", name="at")
                    if exp_bias is not None:
                        nc.scalar.activation(a2[:], pt[:], AF.Exp,
                                             bias=exp_bias[:, sc:sc + 1],
                                             scale=1.0)
                    else:
                        nc.scalar.activation(a2[:], pt[:], AF.Exp,
                                             bias=0.0, scale=1.0)
                    if mask_tiles is not None:
                        for e in range(2):
                            nc.vector.tensor_mul(a2[:, e * TC:e * TC + TC],
                                                 a2[:, e * TC:e * TC + TC],
                                                 mask_tiles[sc][:])
                    if dump and _DEBUG and hp == 0:
                        for e in range(2):
                            dbg_dump(dbg["at"][sc, e],
                                     a2[:, e * TC:e * TC + TC])
                    return a2

                def epilogue_a(hp, pods, den):
                    """approx-recip the two denominator rows (frees den)."""
                    rep = rb_p.tile([P, TC], f32, tag="rep", name="rep")
                    repb = rbb_p.tile([P, TC], bf16, tag="repb", name="repb")
                    if dump and _DEBUG and hp == 0:
                        dbg_dump(dbg["den"][:], den[:, 0:TC])
                        dbg_dump(dbg["pods"][:], pods[:, 0:TC])
                    nc.vector.reciprocal_approx_fast(
                        rep[0:33, :], den[0:33, 0:TC])
                    for e in range(2):
                        r0 = 32 * e
                        nc.vector.tensor_copy(repb[r0:r0 + 1, :],
                                              rep[r0:r0 + 1, :])
                        if dump and _DEBUG and hp == 0:
                            nc.sync.dma_start(dbg["rep"][r0:r0 + 1, :],
                                              rep[r0:r0 + 1, :])
                    return repb

                def epilogue_b(hp, pods, repb):
                    """replicate 1/den across the 64 head partitions via a
                    K=1 matmul, fused normalize+evict into onT."""
                    bc = db_p.tile([P, TC], f32, tag="db", name="bc")
                    bcs = rbb_p.tile([P, TC], bf16, tag="bcs", name="bcs")
                    for e in range(2):
                        r0 = 32 * e
                        nc.tensor.matmul(
                            bc[64 * e:64 * e + DK, 0:TC],
                            ones64[r0:r0 + 1, :],
                            repb[r0:r0 + 1, :],
                            start=True, stop=True,
                            tile_position=(r0, 64 * e),
                            skip_group_check=True)
                    nc.vector.tensor_copy(bcs[:], bc[:, 0:TC])
                    if dump and _DEBUG and hp == 0:
                        dbg_dump(dbg["bcs"][:], bcs[:])
                    for e in range(2):
                        nc.vector.scalar_tensor_tensor(
                            onT[hp][64 * e:64 * e + DK, :],
                            pods[64 * e:64 * e + DK, 0:TC], 0.0,
                            bcs[64 * e:64 * e + DK, :],
                            op0=ALU.bypass, op1=ALU.mult)

                a_store = {}
                for sc in range(NS):
                    a_store[(0, sc)] = score_step(0, sc)
                if after_prologue is not None:
                    after_prologue()
                for hp in range(H // 2):
                    pods = po_p.tile([P, TC], f32, tag="od", name="pods")
                    den = db_p.tile([P, TC], f32, tag="db", name="den")
                    # anchor: one 1x1 matmul waiting on this batch's LAST
                    # a-tile elides the waits on every AV/den matmul below.
                    anchor = a_store[(hp, NS - 1)]
                    nc.tensor.matmul(den[96:97, 0:1], ones_col[0:1, 0:1],
                                     anchor[0:1, 0:1], start=True, stop=True,
                                     tile_position=(0, 96),
                                     skip_group_check=True)
                    for sc in range(NS):
                        a2 = a_store.pop((hp, sc))
                        for e in range(2):
                            h = 2 * hp + e
                            nc.tensor.matmul(
                                pods[64 * e:64 * e + DK, 0:TC],
                                vt[sc][:, h * DK:(h + 1) * DK],
                                a2[:, e * TC:e * TC + TC],
                                start=(sc == 0), stop=(sc == NS - 1),
                                tile_position=(0, 64 * e),
                                skip_group_check=True)
                        for e in range(2):
                            nc.tensor.matmul(
                                den[32 * e:32 * e + 1, 0:TC],
                                ones_col[:, 0:1],
                                a2[:, e * TC:e * TC + TC],
                                start=(sc == 0), stop=(sc == NS - 1),
                                tile_position=(0, 32 * e),
                                skip_group_check=True)
                        if hp + 1 < H // 2:
                            a_store[(hp + 1, sc)] = score_step(hp + 1, sc)
                    repb = epilogue_a(hp, pods, den)
                    epilogue_b(hp, pods, repb)
                if dump and _DEBUG:
                    for k in range(ND):
                        dbg_dump(dbg["on"][k], onT[k][:])

                # out-proj + bias row + residual add into xres (in place)
                for m in range(NT):
                    pt = big2()
                    for n0 in range(0, D, 512):
                        for k in range(ND):
                            nc.tensor.matmul(
                                pt[:, n0:n0 + 512],
                                onT[k][:, m * P:(m + 1) * P],
                                wo_tiles[k][:, n0:n0 + 512],
                                start=(k == 0), stop=False)
                        nc.tensor.matmul(pt[:, n0:n0 + 512], ones_r128[:, 0:P],
                                         c_row[:, n0:n0 + 512],
                                         start=False, stop=True)
                    nc.vector.scalar_tensor_tensor(
                        xres[m][:], pt[:], 0.0, xres[m][:],
                        op0=ALU.bypass, op1=ALU.add)

            def emit():
              # ---------------- self attention ----------------
              # DMA order: xT + wk first so the PE starts ASAP.
              xT_sb = []
              for k in range(ND):
                  t = io.tile([P, T], bf16, tag="xt", name="xt")
                  nc.sync.dma_start(t[0:64, :], xT[k * P:k * P + 64, :])
                  nc.sync.dma_start(t[64:P, :], xT[k * P + 64:(k + 1) * P, :])
                  xT_sb.append(t)
              wk_sb = []
              for k in range(ND):
                  t = wp_p.tile([P, D], bf16, tag="pw", name="pw")
                  nc.sync.dma_start(t[0:64, :], w_sa["k"][k * P:k * P + 64, :])
                  nc.sync.dma_start(t[64:P, :],
                                    w_sa["k"][k * P + 64:(k + 1) * P, :])
                  wk_sb.append(t)
              kt_sa = project_T(wk_sb, xT_sb, ND, kb_sa_sb, "kt", kv_p, S)
              wv_sb = load_w(w_sa["v"], "pw")
              v_sa = project_V(wv_sb, xT_sb, vb_sa_sb, "v")
              # residual stream, fp32, updated in place through the layer
              xres.clear()
              for i in range(NT):
                  t = xres_p.tile([P, D], f32, tag="xres", name="xres")
                  nc.sync.dma_start(t[:], x_res[i * P:(i + 1) * P, :])
                  xres.append(t)
              y1t = layernorm_T(xres, "y1")
              wq_sb = load_w(w_sa["q"], "pw")
              qt_sa = project_T(wq_sb, y1t, 1, qb_sa_sb, "qt", qt_p, TC)
              wo_sb = load_w(w_sa["o"], "pw")

              def _rest_q_sa():
                  project_T(wq_sb, y1t, ND, qb_sa_sb, "qt", qt_p, TC,
                            m_lo=1, otiles=qt_sa)
              # mask tiles (self-attn only): duplicated into both halves so
              # the fused [P, 2*TC] mask multiply covers the score pair
              mk = []
              for i in range(NS):
                  t = mask_p.tile([P, TC], bf16, tag="mk", name="mk")
                  nc.sync.dma_start(t[:], maskT[i * P:(i + 1) * P, :])
                  mk.append(t)
              attention(kt_sa, v_sa, qt_sa, None, mk, wo_sb, c_sa_sb,
                        dump=True, after_prologue=_rest_q_sa)

              # ---------------- cross attention ----------------
              memT_sb = []
              for k in range(ND):
                  t = io.tile([P, S], bf16, tag="xt", name="xt")
                  nc.sync.dma_start(t[:], memT[k * P:(k + 1) * P, :])
                  memT_sb.append(t)
              wk_sb = load_w(w_ca["k"], "pw")
              kt_ca = project_T(wk_sb, memT_sb, ND, kb_ca_sb, "kt", kv_p, S)
              wv_sb = load_w(w_ca["v"], "pw")
              v_ca = project_V(wv_sb, memT_sb, vb_ca_sb, "v")
              y2t = layernorm_T(xres, "y2")
              wq_sb = load_w(w_ca["q"], "pw")
              qt_ca = project_T(wq_sb, y2t, 1, qb_ca_sb, "qt", qt_p, TC)
              wo_sb = load_w(w_ca["o"], "pw")

              def _rest_q_ca():
                  project_T(wq_sb, y2t, ND, qb_ca_sb, "qt", qt_p, TC,
                            m_lo=1, otiles=qt_ca)
              attention(kt_ca, v_ca, qt_ca, sb_ca_sb, None, wo_sb, c_ca_sb,
                        after_prologue=_rest_q_ca)

              # ---------------- FFN ----------------
              y3t = layernorm_T(xres, "y3")
              h1 = []                       # (tile, col offset) pairs
              for fg in range(8):          # 8 groups of 4 F-tiles
                  w1g = []
                  for k in range(ND):
                      t = w1_p.tile([P, 512], bf16, tag="w1", name="w1")
                      nc.sync.dma_start(
                          t[:], w1T[k * P:(k + 1) * P, fg * 512:(fg + 1) * 512])
                      w1g.append(t)
                  for fj2 in range(2):     # one big2 psum holds 2 fj chunks
                      pt = big2()
                      ht = at_p.tile([P, 2 * TC], bf16, tag="at", name="h1")
                      for j in range(2):
                          fj = fj2 * 2 + j
                          fm = fg * 4 + fj
                          for k in range(ND):
                              nc.tensor.matmul(
                                  pt[:, j * TC:j * TC + TC],
                                  w1g[k][:, fj * P:(fj + 1) * P],
                                  y3t[k][:], start=(k == 0),
                                  stop=(k == ND - 1))
                          nc.scalar.activation(ht[:, j * TC:j * TC + TC],
                                               pt[:, j * TC:j * TC + TC],
                                               AF.Relu,
                                               bias=h1b_sb[:, fm:fm + 1],
                                               scale=1.0)
                          h1.append((ht, j * TC))
              for n0 in range(0, D, 512):
                  pts = [big2() for _ in range(2)]
                  for f in range(NF):
                      wt = w2_p.tile([P, 512], bf16, tag="w2", name="w2")
                      nc.sync.dma_start(
                          wt[:], w2T[f * P:(f + 1) * P, n0:n0 + 512])
                      ht, off = h1[f]
                      for m in range(NT):
                          nc.tensor.matmul(
                              pts[m // 2][:, (m % 2) * TC:(m % 2) * TC + TC],
                              ht[:, off + m * P:off + (m + 1) * P], wt[:],
                              start=(f == 0), stop=False)
                  for m in range(NT):
                      sl = pts[m // 2][:, (m % 2) * TC:(m % 2) * TC + TC]
                      nc.tensor.matmul(sl, ones_r128[:, 0:P],
                                       c_ffn_sb[:, n0:n0 + 512],
                                       start=False, stop=True)
                      nc.vector.scalar_tensor_tensor(
                          xres[m][:, n0:n0 + 512], sl, 0.0,
                          xres[m][:, n0:n0 + 512],
                          op0=ALU.bypass, op1=ALU.add)

              # ---------------- write out ----------------
              for m in range(NT):
                  nc.sync.dma_start(out[m * P:(m + 1) * P, :], xres[m][:])

            xres = []
            for _rep in range(repeat):
                emit()

    nc.compile()
    return nc


def _prep_inputs(inputs):
    from concourse import mybir
    bf16 = mybir.dt.np(mybir.dt.bfloat16)

    f = {k: np.asarray(v, dtype=np.float32) for k, v in inputs.items()
         if k not in ("trg_mask", "trg_causal_mask", "src_mask")}
    trg_mask = np.asarray(inputs["trg_mask"])          # [B,1,1,T] int32
    causal = np.asarray(inputs["trg_causal_mask"])     # [1,1,T,T] int32
    src_mask = np.asarray(inputs["src_mask"])          # [B,1,1,S] int32

    def bf(a):
        return np.ascontiguousarray(a.astype(np.float32)).astype(bf16)

    def fold_cols(v):      # [N] -> [128, N/128], col a = v[a*128:(a+1)*128]
        return np.ascontiguousarray(v.reshape(-1, P).T.astype(np.float32))

    scale = 1.0 / np.sqrt(np.float32(DK))
    shared = {
        "wq_sa": bf((f["sa_wq"] * f["ln1_g"][None, :] * scale).T),
        "wk_sa": bf(f["sa_wk"].T),
        "wv_sa": bf(f["sa_wv"].T),
        "wo_sa": bf(f["sa_wo"].T),
        "wq_ca": bf((f["ca_wq"] * f["ln2_g"][None, :] * scale).T),
        "wk_ca": bf(f["ca_wk"].T),
        "wv_ca": bf(f["ca_wv"].T),
        "wo_ca": bf(f["ca_wo"].T),
        "qb_sa": fold_cols((f["ln1_b"] @ f["sa_wq"].T + f["sa_bq"]) * scale),
        "kb_sa": fold_cols(f["sa_bk"]),
        "qb_ca": fold_cols((f["ln2_b"] @ f["ca_wq"].T + f["ca_bq"]) * scale),
        "kb_ca": fold_cols(f["ca_bk"]),
        "h1b": fold_cols(f["ln3_b"] @ f["ffn_w1"].T + f["ffn_b1"]),
        "vb_sa": bf(f["sa_bv"][None, :]),
        "vb_ca": bf(f["ca_bv"][None, :]),
        "c_sa": bf(f["sa_bo"][None, :]),
        "c_ca": bf(f["ca_bo"][None, :]),
        "c_ffn": bf(f["ffn_b2"][None, :]),
        "w1T": bf((f["ffn_w1"] * f["ln3_g"][None, :]).T),
        "w2T": bf(f["ffn_w2"].T),
    }

    # allowed[t, s] = causal[t, s] & trg_mask[b, s]; transposed -> [s, t]
    allowed = (causal[0, 0] != 0).astype(np.float32)        # [T, T]
    in_maps = []
    for c in range(NCORES):
        b, h = c // 2, c % 2
        rows = slice(h * TC, (h + 1) * TC)
        m_b = allowed * (trg_mask[b, 0, 0, :] != 0).astype(np.float32)[None, :]
        sb = (np.float32(f["ca_scale"]) * f["sentence_bias"][b]
              + np.where(src_mask[b, 0, 0, :] != 0, 0.0, NEG).astype(np.float32))
        im = dict(shared)
        im["xT"] = bf(f["x"][b].T)
        im["memT"] = bf(f["memory"][b].T)
        im["x_res"] = np.ascontiguousarray(f["x"][b, rows])
        im["maskT"] = bf(m_b[rows, :].T)
        im["sb_ca"] = fold_cols(sb)
        in_maps.append(im)
    return in_maps


def kernel(**inputs):
    from concourse.bass_utils import run_bass_kernel_spmd

    if "nc" not in _CACHE:
        _CACHE["nc"] = _build()
    nc = _CACHE["nc"]

    in_maps = _prep_inputs(inputs)
    res = run_bass_kernel_spmd(nc, in_maps, core_ids=list(range(NCORES)))

    full = np.empty((B, T, D), np.float32)
    for c in range(NCORES):
        b, h = c // 2, c % 2
        full[b, h * TC:(h + 1) * TC, :] = res.results[c]["out"]
    return full


# revision 41
# speedup vs baseline: 5.6532x; 1.0050x over previous
"""Trainium2 Bass kernel for nn_DecoderLayer (B=4, T=S=1024, D=1024, H=16, F=4096).

Sharding: 8 cores = batch (4) x sequence-half (2). Each core computes 512 output
rows of one batch. Self-attn K/V come from raw x (full batch, host-provided
transposed), cross-attn K/V from memory -- so no inter-core communication is
needed; the host scatters inputs and gathers the 8 output chunks.

Per-core dataflow (bf16 matmuls, fp32 accumulation / residual stream):
  - K^T = wk @ x^T from host-provided x^T (contraction dim D on partitions
    everywhere; no on-device transposes of big tensors). Q^T = wq_eff @
    ln(x)^T with LN gain and 1/sqrt(dk) folded into wq host-side.
  - PSUM discipline: all large psum tiles are [128, 1024] spanning two banks
    ("big2"), so paired matmuls (score head-pairs, projection column chunks,
    FFN m-pairs) write the two halves of ONE ring slot. The pair then shares
    one ring dependency, whose wait the Tile scheduler elides for the second
    matmul (a matmul carrying its own semaphore wait runs isolated, ~535 ns
    vs ~330 ns pipelined for N=512).
  - Attention is hp-batched: in batch(hp), the A^T@V / denominator matmuls
    consume exp'd score tiles produced a full batch earlier, and a 1x1
    "anchor" matmul at the batch head waits on the batch's LAST a-tile --
    all following AV/den waits are elided and the sub-tile matmul pairs
    (AV: M=64 at column groups 0/64; den: M=1 at 0/32) run concurrently.
    Scores for batch hp+1 interleave, paced by the ACT exp stream.
  - Softmax is max-free (logits are O(1) for this model family); exp+mask
    run as ONE wide ACT/DVE op over the [128, 1024] score pair; denominator
    reciprocal via the fast approx DVE op, replicated across the 64 head
    partitions with a K=1 matmul, then fused normalize+evict.
  - Free-dim biases enter as K=1 ones-row matmul terms; per-partition biases
    ride (fused, 1024-wide) activation eviction.
  - FFN: h1^T = relu(w1_eff @ ln(x)^T + b) -> h2 = h1^T.T @ w2^T, h1 tiles
    packed two-per [128, 1024] SBUF tile, w2 accumulation into big2 halves.
"""

import sys

if "/opt/trn_rl_repo" not in sys.path:
    sys.path.insert(0, "/opt/trn_rl_repo")

import numpy as np

B, T, S, D, H, F = 4, 1024, 1024, 1024, 16, 4096
DK = D // H          # 64
P = 128
NCORES = 8
TC = T // 2          # 512 rows per core
NT = TC // P         # 4 row tiles per core
ND = D // P          # 8
NS = S // P          # 8
NF = F // P          # 32
NEG = np.float32(-1e9)

_CACHE = {}
_DEBUG = False


def _build(repeat=1):
    import concourse.bacc as bacc
    import concourse.bass as bass
    import concourse.tile as tile
    from concourse import mybir
    from concourse.masks import make_identity

    f32 = mybir.dt.float32
    bf16 = mybir.dt.bfloat16
    AF = mybir.ActivationFunctionType
    ALU = mybir.AluOpType

    nc = bacc.Bacc("TRN2", target_bir_lowering=False, debug=False,
                   num_devices=NCORES)

    # ---------------- DRAM I/O ----------------
    dt_in = {}

    def din(name, shape, dt):
        dt_in[name] = nc.dram_tensor(name, list(shape), dt, kind="ExternalInput")
        return dt_in[name]

    xT = din("xT", (D, T), bf16)            # x[b].T
    memT = din("memT", (D, S), bf16)        # memory[b].T
    x_res = din("x_res", (TC, D), f32)      # x[b, rows]  (residual stream)
    maskT = din("maskT", (S, TC), bf16)     # binary allowed-mask, transposed
    sb_ca = din("sb_ca", (P, NS), f32)      # cross exp bias per key position
    qb_sa = din("qb_sa", (P, ND), f32)      # per-partition bias for Q^T (self)
    kb_sa = din("kb_sa", (P, ND), f32)
    qb_ca = din("qb_ca", (P, ND), f32)
    kb_ca = din("kb_ca", (P, ND), f32)
    h1b = din("h1b", (P, NF), f32)          # per-partition bias for h1^T
    w_sa = {k: din(f"w{k}_sa", (D, D), bf16) for k in ("q", "k", "v", "o")}
    w_ca = {k: din(f"w{k}_ca", (D, D), bf16) for k in ("q", "k", "v", "o")}
    vb_sa = din("vb_sa", (1, D), bf16)      # V-proj bias row (ones-row matmul)
    vb_ca = din("vb_ca", (1, D), bf16)
    c_sa = din("c_sa", (1, D), bf16)        # out-proj bias row
    c_ca = din("c_ca", (1, D), bf16)
    c_ffn = din("c_ffn", (1, D), bf16)      # ffn_b2 row
    w1T = din("w1T", (D, F), bf16)
    w2T = din("w2T", (F, D), bf16)
    out = nc.dram_tensor("out", [TC, D], f32, kind="ExternalOutput")
    dbg = {}
    if _DEBUG:
        dbg["at"] = nc.dram_tensor("dbg_at", [NS, 2, P, TC], f32, kind="ExternalOutput")
        dbg["den"] = nc.dram_tensor("dbg_den", [P, TC], f32, kind="ExternalOutput")
        dbg["pods"] = nc.dram_tensor("dbg_pods", [P, TC], f32, kind="ExternalOutput")
        dbg["rep"] = nc.dram_tensor("dbg_rep", [P, TC], f32, kind="ExternalOutput")
        dbg["bcs"] = nc.dram_tensor("dbg_bcs", [P, TC], f32, kind="ExternalOutput")
        dbg["on"] = nc.dram_tensor("dbg_on", [ND, P, TC], f32, kind="ExternalOutput")

    from contextlib import ExitStack

    with tile.TileContext(nc) as tc:
        with ExitStack() as ctx:
            pool = lambda name, bufs, **kw: ctx.enter_context(
                tc.tile_pool(name=name, bufs=bufs, **kw))
            const = pool("const", 1)
            io = pool("io", 8)
            xres_p = pool("xres", 4)
            kv_p = pool("kv", 8)
            qt_p = pool("qt", 8)
            at_p = pool("at", 16)           # [P, 2*TC] a-tiles / h1 pairs
            ot_p = pool("ot", 8)
            yy_p = pool("yy", 3)
            yt_p = pool("yt", 8)
            wp_p = pool("wp", 16)
            w1_p = pool("w1p", 8)
            w2_p = pool("w2p", 8)
            mask_p = pool("mask", 8)
            sm_p = pool("sm", 16)
            rb_p = pool("rb", 2)
            rbb_p = pool("rbb", 2)
            dbg_p = pool("dbg", 2) if _DEBUG else None
            # PSUM: 8 banks: big2 2x2 + pods 2 + den/bc shared 2
            ps_p = pool("ps", 2, space="PSUM")
            po_p = pool("po", 2, space="PSUM")
            db_p = pool("db", 2, space="PSUM")

            def big2():
                return ps_p.tile([P, 2 * TC], f32, tag="big2", name="big2")

            # ---------------- constants ----------------
            ident = const.tile([P, P], bf16)
            make_identity(nc, ident[:])
            ones_col = const.tile([P, 1], bf16)
            nc.vector.memset(ones_col[:], 1.0)
            ones64 = const.tile([P, 64], bf16)
            nc.vector.memset(ones64[:], 1.0)
            ones_r128 = const.tile([1, P], bf16)
            nc.vector.memset(ones_r128[:], 1.0)
            eps = const.tile([P, 1], f32)
            nc.vector.memset(eps[:], 1e-5)

            def load_const(name, shape, dt):
                t = const.tile(list(shape), dt, tag=name, name=name)
                nc.sync.dma_start(t[:], dt_in[name][:])
                return t

            sb_ca_sb = load_const("sb_ca", (P, NS), f32)
            qb_sa_sb = load_const("qb_sa", (P, ND), f32)
            kb_sa_sb = load_const("kb_sa", (P, ND), f32)
            qb_ca_sb = load_const("qb_ca", (P, ND), f32)
            kb_ca_sb = load_const("kb_ca", (P, ND), f32)
            h1b_sb = load_const("h1b", (P, NF), f32)
            vb_sa_sb = load_const("vb_sa", (1, D), bf16)
            vb_ca_sb = load_const("vb_ca", (1, D), bf16)
            c_sa_sb = load_const("c_sa", (1, D), bf16)
            c_ca_sb = load_const("c_ca", (1, D), bf16)
            c_ffn_sb = load_const("c_ffn", (1, D), bf16)

            # ---------------- helpers ----------------
            def load_w(dram, tag, pool=wp_p, width=D):
                tiles = []
                for k in range(ND):
                    t = pool.tile([P, width], bf16, tag=tag, name=tag)
                    nc.sync.dma_start(t[:], dram[k * P:(k + 1) * P, :])
                    tiles.append(t)
                return tiles

            def layernorm_T(src_tiles, tag):
                """LN (stats only; gain/bias folded into weights downstream) of
                the fp32 [TC, D] residual -> bf16 normalized rows, then PE
                transpose -> yt tiles [P, TC] (D on partitions)."""
                ytiles = [yt_p.tile([P, TC], bf16, tag="yt", name="yt")
                          for _ in range(ND)]
                for i in range(NT):
                    xt = src_tiles[i]
                    stats = sm_p.tile([P, 2, 6], f32, tag="stats", name="stats")
                    mv = sm_p.tile([P, 2], f32, tag="mv", name="mv")
                    nc.vector.bn_stats(stats[:, 0, :], xt[:, 0:512])
                    nc.vector.bn_stats(stats[:, 1, :], xt[:, 512:1024])
                    nc.vector.bn_aggr(mv[:], stats[:])
                    rstd = sm_p.tile([P, 1], f32, tag="rstd", name="rstd")
                    nc.scalar.activation(rstd[:], mv[:, 1:2], AF.Sqrt,
                                         bias=eps[:], scale=1.0)
                    nc.vector.reciprocal(rstd[:], rstd[:])
                    negmr = sm_p.tile([P, 1], f32, tag="negmr", name="negmr")
                    nc.vector.scalar_tensor_tensor(
                        negmr[:], mv[:, 0:1], -1.0, rstd[:],
                        op0=ALU.mult, op1=ALU.mult)
                    xhat = yy_p.tile([P, D], bf16, tag="xhat", name="xhat")
                    nc.scalar.activation(xhat[:], xt[:], AF.Identity,
                                         bias=negmr[:], scale=rstd[:])
                    for d in range(ND):
                        pt = ps_p.tile([P, P], bf16, tag="big2", name="pstp")
                        nc.tensor.transpose(pt[:],
                                            xhat[:, d * P:(d + 1) * P],
                                            ident[:])
                        nc.vector.tensor_copy(
                            ytiles[d][:, i * P:(i + 1) * P], pt[:])
                return ytiles

            def project_T(w_tiles, rhs_tiles, n_out, bias_sb, out_tag,
                          out_pool, width, m_lo=0, otiles=None):
                """out^T[o, n] = w^T.T @ rhs  (contraction over D on
                partitions). rhs_tiles: ND tiles [P, width]. Output: n_out
                tiles [P, width] bf16, fused ACT eviction with per-partition
                bias."""
                if otiles is None:
                    otiles = []
                for m in range(m_lo, n_out):
                    ot = out_pool.tile([P, width], bf16, tag=out_tag, name=out_tag)
                    pt = big2()
                    for n0 in range(0, width, 512):
                        for k in range(ND):
                            nc.tensor.matmul(
                                pt[:, n0:n0 + 512],
                                w_tiles[k][:, m * P:(m + 1) * P],
                                rhs_tiles[k][:, n0:n0 + 512],
                                start=(k == 0), stop=(k == ND - 1))
                    nc.scalar.activation(ot[:, 0:width], pt[:, 0:width],
                                         AF.Identity,
                                         bias=bias_sb[:, m:m + 1],
                                         scale=1.0)
                    otiles.append(ot)
                return otiles

            def project_V(w_tiles, src_tiles, vb_row, out_tag):
                """V[s, d] tiles (key positions on partitions). vb enters via
                a K=1 ones-row matmul term; fused eviction on DVE."""
                vtiles = []
                for m in range(NS):
                    vt = kv_p.tile([P, D], bf16, tag=out_tag, name=out_tag)
                    pt = big2()
                    for n0 in range(0, D, 512):
                        for k in range(ND):
                            nc.tensor.matmul(
                                pt[:, n0:n0 + 512],
                                src_tiles[k][:, m * P:(m + 1) * P],
                                w_tiles[k][:, n0:n0 + 512],
                                start=(k == 0), stop=False)
                        nc.tensor.matmul(pt[:, n0:n0 + 512], ones_r128[:, 0:P],
                                         vb_row[:, n0:n0 + 512],
                                         start=False, stop=True)
                    nc.vector.tensor_copy(vt[:], pt[:])
                    vtiles.append(vt)
                return vtiles

            def dbg_dump(dram_slice, src_ap):
                t = dbg_p.tile([P, TC], mybir.dt.float32, tag="dbg", name="dbg")
                nc.vector.tensor_copy(t[:], src_ap)
                nc.sync.dma_start(dram_slice, t[:])

            def attention(kt, vt, qt, exp_bias, mask_tiles, wo_tiles, c_row,
                          dump=False, after_prologue=None):
                """hp-batched transposed-scores attention (see module doc).
                Adds attn output + c_row into xres in place via wo."""
                onT = [ot_p.tile([P, TC], bf16, tag="onT", name="onT")
                       for _ in range(ND)]

                def score_step(hp, sc):
                    """score pair (one big2 psum) -> fused exp+mask -> a2."""
                    pt = big2()
                    for e in range(2):
                        po = 64 * e
                        nc.tensor.matmul(
                            pt[:, e * TC:e * TC + TC],
                            kt[hp][po:po + DK, sc * P:(sc + 1) * P],
                            qt[hp][po:po + DK, :],
                            start=True, stop=True)
                    a2 = at_p.tile([P, 2 * TC], bf16, tag="at", name="at")
                    if exp_bias is not None:
                        nc.scalar.activation(a2[:], pt[:], AF.Exp,
                                             bias=exp_bias[:, sc:sc + 1],
                                             scale=1.0)
                    else:
                        nc.scalar.activation(a2[:], pt[:], AF.Exp,
                                             bias=0.0, scale=1.0)
                    if mask_tiles is not None:
                        for e in range(2):
                            nc.vector.tensor_mul(a2[:, e * TC:e * TC + TC],
                                                 a2[:, e * TC:e * TC + TC],
                                                 mask_tiles[sc][:])
                    if dump and _DEBUG and hp == 0:
                        for e in range(2):
                            dbg_dump(dbg["at"][sc, e],
                                     a2[:, e * TC:e * TC + TC])
                    return a2

                def epilogue_a(hp, pods, den):
                    """approx-recip the two denominator rows (frees den)."""
                    rep = rb_p.tile([P, TC], f32, tag="rep", name="rep")
                    repb = rbb_p.tile([P, TC], bf16, tag="repb", name="repb")
                    if dump and _DEBUG and hp == 0:
                        dbg_dump(dbg["den"][:], den[:, 0:TC])
                        dbg_dump(dbg["pods"][:], pods[:, 0:TC])
                    nc.vector.reciprocal_approx_fast(
                        rep[0:33, :], den[0:33, 0:TC])
                    for e in range(2):
                        r0 = 32 * e
                        nc.vector.tensor_copy(repb[r0:r0 + 1, :],
                                              rep[r0:r0 + 1, :])
                        if dump and _DEBUG and hp == 0:
                            nc.sync.dma_start(dbg["rep"][r0:r0 + 1, :],
                                              rep[r0:r0 + 1, :])
                    return repb

                def epilogue_b(hp, pods, repb):
                    """replicate 1/den across the 64 head partitions via a
                    K=1 matmul, fused normalize+evict into onT."""
                    bc = db_p.tile([P, TC], f32, tag="db", name="bc")
                    bcs = rbb_p.tile([P, TC], bf16, tag="bcs", name="bcs")
                    for e in range(2):
                        r0 = 32 * e
                        nc.tensor.matmul(
                            bc[64 * e:64 * e + DK, 0:TC],
                            ones64[r0:r0 + 1, :],
                            repb[r0:r0 + 1, :],
                            start=True, stop=True,
                            tile_position=(r0, 64 * e),
                            skip_group_check=True)
                    nc.vector.tensor_copy(bcs[:], bc[:, 0:TC])
                    if dump and _DEBUG and hp == 0:
                        dbg_dump(dbg["bcs"][:], bcs[:])
                    for e in range(2):
                        nc.vector.scalar_tensor_tensor(
                            onT[hp][64 * e:64 * e + DK, :],
                            pods[64 * e:64 * e + DK, 0:TC], 0.0,
                            bcs[64 * e:64 * e + DK, :],
                            op0=ALU.bypass, op1=ALU.mult)

                a_store = {}
                for sc in range(NS):
                    a_store[(0, sc)] = score_step(0, sc)
                if after_prologue is not None:
                    after_prologue()
                for hp in range(H // 2):
                    pods = po_p.tile([P, TC], f32, tag="od", name="pods")
                    den = db_p.tile([P, TC], f32, tag="db", name="den")
                    # anchor: one 1x1 matmul waiting on this batch's LAST
                    # a-tile elides the waits on every AV/den matmul below.
                    anchor = a_store[(hp, NS - 1)]
                    nc.tensor.matmul(den[96:97, 0:1], ones_col[0:1, 0:1],
                                     anchor[0:1, 0:1], start=True, stop=True,
                                     tile_position=(0, 96),
                                     skip_group_check=True)
                    for sc in range(NS):
                        a2 = a_store.pop((hp, sc))
                        for e in range(2):
                            h = 2 * hp + e
                            nc.tensor.matmul(
                                pods[64 * e:64 * e + DK, 0:TC],
                                vt[sc][:, h * DK:(h + 1) * DK],
                                a2[:, e * TC:e * TC + TC],
                                start=(sc == 0), stop=(sc == NS - 1),
                                tile_position=(0, 64 * e),
                                skip_group_check=True)
                        for e in range(2):
                            nc.tensor.matmul(
                                den[32 * e:32 * e + 1, 0:TC],
                                ones_col[:, 0:1],
                                a2[:, e * TC:e * TC + TC],
                                start=(sc == 0), stop=(sc == NS - 1),
                                tile_position=(0, 32 * e),
                                skip_group_check=True)
                        if hp + 1 < H // 2:
                            a_store[(hp + 1, sc)] = score_step(hp + 1, sc)
                    repb = epilogue_a(hp, pods, den)
                    epilogue_b(hp, pods, repb)
                if dump and _DEBUG:
                    for k in range(ND):
                        dbg_dump(dbg["on"][k], onT[k][:])

                # out-proj + bias row + residual add into xres (in place)
                for m in range(NT):
                    pt = big2()
                    for n0 in range(0, D, 512):
                        for k in range(ND):
                            nc.tensor.matmul(
                                pt[:, n0:n0 + 512],
                                onT[k][:, m * P:(m + 1) * P],
                                wo_tiles[k][:, n0:n0 + 512],
                                start=(k == 0), stop=False)
                        nc.tensor.matmul(pt[:, n0:n0 + 512], ones_r128[:, 0:P],
                                         c_row[:, n0:n0 + 512],
                                         start=False, stop=True)
                    nc.vector.scalar_tensor_tensor(
                        xres[m][:], pt[:], 0.0, xres[m][:],
                        op0=ALU.bypass, op1=ALU.add)

            def emit():
              # ---------------- self attention ----------------
              # DMA order: xT + wk first so the PE starts ASAP.
              xT_sb = []
              for k in range(ND):
                  t = io.tile([P, T], bf16, tag="xt", name="xt")
                  nc.sync.dma_start(t[0:64, :], xT[k * P:k * P + 64, :])
                  nc.sync.dma_start(t[64:P, :], xT[k * P + 64:(k + 1) * P, :])
                  xT_sb.append(t)
              wk_sb = []
              for k in range(ND):
                  t = wp_p.tile([P, D], bf16, tag="pw", name="pw")
                  nc.sync.dma_start(t[0:64, :], w_sa["k"][k * P:k * P + 64, :])
                  nc.sync.dma_start(t[64:P, :],
                                    w_sa["k"][k * P + 64:(k + 1) * P, :])
                  wk_sb.append(t)
              kt_sa = project_T(wk_sb, xT_sb, ND, kb_sa_sb, "kt", kv_p, S)
              wv_sb = load_w(w_sa["v"], "pw")
              v_sa = project_V(wv_sb, xT_sb, vb_sa_sb, "v")
              # residual stream, fp32, updated in place through the layer
              xres.clear()
              for i in range(NT):
                  t = xres_p.tile([P, D], f32, tag="xres", name="xres")
                  nc.sync.dma_start(t[:], x_res[i * P:(i + 1) * P, :])
                  xres.append(t)
              y1t = layernorm_T(xres, "y1")
              wq_sb = load_w(w_sa["q"], "pw")
              qt_sa = project_T(wq_sb, y1t, 1, qb_sa_sb, "qt", qt_p, TC)
              wo_sb = load_w(w_sa["o"], "pw")

              def _rest_q_sa():
                  project_T(wq_sb, y1t, ND, qb_sa_sb, "qt", qt_p, TC,
                            m_lo=1, otiles=qt_sa)
              # mask tiles (self-attn only): duplicated into both halves so
              # the fused [P, 2*TC] mask multiply covers the score pair
              mk = []
              for i in range(NS):
                  t = mask_p.tile([P, TC], bf16, tag="mk", name="mk")
                  nc.sync.dma_start(t[:], maskT[i * P:(i + 1) * P, :])
                  mk.append(t)
              attention(kt_sa, v_sa, qt_sa, None, mk, wo_sb, c_sa_sb,
                        dump=True, after_prologue=_rest_q_sa)

              # ---------------- cross attention ----------------
              memT_sb = []
              for k in range(ND):
                  t = io.tile([P, S], bf16, tag="xt", name="xt")
                  nc.sync.dma_start(t[:], memT[k * P:(k + 1) * P, :])
                  memT_sb.append(t)
              wk_sb = load_w(w_ca["k"], "pw")
              kt_ca = project_T(wk_sb, memT_sb, ND, kb_ca_sb, "kt", kv_p, S)
              wv_sb = load_w(w_ca["v"], "pw")
              v_ca = project_V(wv_sb, memT_sb, vb_ca_sb, "v")
              y2t = layernorm_T(xres, "y2")
              wq_sb = load_w(w_ca["q"], "pw")
              qt_ca = project_T(wq_sb, y2t, 1, qb_ca_sb, "qt", qt_p, TC)
              wo_sb = load_w(w_ca["o"], "pw")

              def _rest_q_ca():
                  project_T(wq_sb, y2t, ND, qb_ca_sb, "qt", qt_p, TC,
                            m_lo=1, otiles=qt_ca)
              attention(kt_ca, v_ca, qt_ca, sb_ca_sb, None, wo_sb, c_ca_sb,
                        after_prologue=_rest_q_ca)

              # ---------------- FFN ----------------
              y3t = layernorm_T(xres, "y3")
              h1 = []                       # (tile, col offset) pairs
              for fg in range(8):          # 8 groups of 4 F-tiles
                  w1g = []
                  for k in range(ND):
                      t = w1_p.tile([P, 512], bf16, tag="w1", name="w1")
                      nc.sync.dma_start(
                          t[:], w1T[k * P:(k + 1) * P, fg * 512:(fg + 1) * 512])
                      w1g.append(t)
                  for fj2 in range(2):     # one big2 psum holds 2 fj chunks
                      pt = big2()
                      ht = at_p.tile([P, 2 * TC], bf16, tag="at", name="h1")
                      for j in range(2):
                          fj = fj2 * 2 + j
                          fm = fg * 4 + fj
                          for k in range(ND):
                              nc.tensor.matmul(
                                  pt[:, j * TC:j * TC + TC],
                                  w1g[k][:, fj * P:(fj + 1) * P],
                                  y3t[k][:], start=(k == 0),
                                  stop=(k == ND - 1))
                          nc.scalar.activation(ht[:, j * TC:j * TC + TC],
                                               pt[:, j * TC:j * TC + TC],
                                               AF.Relu,
                                               bias=h1b_sb[:, fm:fm + 1],
                                               scale=1.0)
                          h1.append((ht, j * TC))
              for n0 in range(0, D, 512):
                  pts = [big2() for _ in range(2)]
                  for f in range(NF):
                      wt = w2_p.tile([P, 512], bf16, tag="w2", name="w2")
                      nc.sync.dma_start(
                          wt[:], w2T[f * P:(f + 1) * P, n0:n0 + 512])
                      ht, off = h1[f]
                      for m in range(NT):
                          nc.tensor.matmul(
                              pts[m // 2][:, (m % 2) * TC:(m % 2) * TC + TC],
                              ht[:, off + m * P:off + (m + 1) * P], wt[:],
                              start=(f == 0), stop=False)
                  for m in range(NT):
                      sl = pts[m // 2][:, (m % 2) * TC:(m % 2) * TC + TC]
                      nc.tensor.matmul(sl, ones_r128[:, 0:P],
                                       c_ffn_sb[:, n0:n0 + 512],
                                       start=False, stop=True)
                      nc.vector.scalar_tensor_tensor(
                          xres[m][:, n0:n0 + 512], sl, 0.0,
                          xres[m][:, n0:n0 + 512],
                          op0=ALU.bypass, op1=ALU.add)

              # ---------------- write out ----------------
              for m in range(NT):
                  nc.sync.dma_start(out[m * P:(m + 1) * P, :], xres[m][:])

            xres = []
            for _rep in range(repeat):
                emit()

    nc.compile()
    return nc


def _prep_inputs(inputs):
    from concourse import mybir
    bf16 = mybir.dt.np(mybir.dt.bfloat16)

    f = {k: np.asarray(v, dtype=np.float32) for k, v in inputs.items()
         if k not in ("trg_mask", "trg_causal_mask", "src_mask")}
    trg_mask = np.asarray(inputs["trg_mask"])          # [B,1,1,T] int32
    causal = np.asarray(inputs["trg_causal_mask"])     # [1,1,T,T] int32
    src_mask = np.asarray(inputs["src_mask"])          # [B,1,1,S] int32

    def bf(a):
        return np.ascontiguousarray(a.astype(np.float32)).astype(bf16)

    def fold_cols(v):      # [N] -> [128, N/128], col a = v[a*128:(a+1)*128]
        return np.ascontiguousarray(v.reshape(-1, P).T.astype(np.float32))

    scale = 1.0 / np.sqrt(np.float32(DK))
    shared = {
        "wq_sa": bf((f["sa_wq"] * f["ln1_g"][None, :] * scale).T),
        "wk_sa": bf(f["sa_wk"].T),
        "wv_sa": bf(f["sa_wv"].T),
        "wo_sa": bf(f["sa_wo"].T),
        "wq_ca": bf((f["ca_wq"] * f["ln2_g"][None, :] * scale).T),
        "wk_ca": bf(f["ca_wk"].T),
        "wv_ca": bf(f["ca_wv"].T),
        "wo_ca": bf(f["ca_wo"].T),
        "qb_sa": fold_cols((f["ln1_b"] @ f["sa_wq"].T + f["sa_bq"]) * scale),
        "kb_sa": fold_cols(f["sa_bk"]),
        "qb_ca": fold_cols((f["ln2_b"] @ f["ca_wq"].T + f["ca_bq"]) * scale),
        "kb_ca": fold_cols(f["ca_bk"]),
        "h1b": fold_cols(f["ln3_b"] @ f["ffn_w1"].T + f["ffn_b1"]),
        "vb_sa": bf(f["sa_bv"][None, :]),
        "vb_ca": bf(f["ca_bv"][None, :]),
        "c_sa": bf(f["sa_bo"][None, :]),
        "c_ca": bf(f["ca_bo"][None, :]),
        "c_ffn": bf(f["ffn_b2"][None, :]),
        "w1T": bf((f["ffn_w1"] * f["ln3_g"][None, :]).T),
        "w2T": bf(f["ffn_w2"].T),
    }

    # allowed[t, s] = causal[t, s] & trg_mask[b, s]; transposed -> [s, t]
    allowed = (causal[0, 0] != 0).astype(np.float32)        # [T, T]
    in_maps = []
    for c in range(NCORES):
        b, h = c // 2, c % 2
        rows = slice(h * TC, (h + 1) * TC)
        m_b = allowed * (trg_mask[b, 0, 0, :] != 0).astype(np.float32)[None, :]
        sb = (np.float32(f["ca_scale"]) * f["sentence_bias"][b]
              + np.where(src_mask[b, 0, 0, :] != 0, 0.0, NEG).astype(np.float32))
        im = dict(shared)
        im["xT"] = bf(f["x"][b].T)
        im["memT"] = bf(f["memory"][b].T)
        im["x_res"] = np.ascontiguousarray(f["x"][b, rows])
        im["maskT"] = bf(m_b[rows, :].T)
        im["sb_ca"] = fold_cols(sb)
        in_maps.append(im)
    return in_maps


def kernel(**inputs):
    from concourse.bass_utils import run_bass_kernel_spmd

    if "nc" not in _CACHE:
        _CACHE["nc"] = _build()
    nc = _CACHE["nc"]

    in_maps = _prep_inputs(inputs)
    res = run_bass_kernel_spmd(nc, in_maps, core_ids=list(range(NCORES)))

    full = np.empty((B, T, D), np.float32)
    for c in range(NCORES):
        b, h = c // 2, c % 2
        full[b, h * TC:(h + 1) * TC, :] = res.results[c]["out"]
    return full
